# revision 4
# baseline (speedup 1.0000x reference)
"""Trainium2 Bass kernel for additive attention (nn_Attention).

Reference computation (B=32, S=4096, D=512):
    q_proj = query @ W1 + b1                      # [B, D]
    v_proj = values @ W2 + b2                     # [B, S, D]
    hidden = tanh(q_proj[:, None, :] + v_proj)    # [B, S, D]
    score  = hidden @ V + bv                      # [B, S, 1]
    attn   = softmax(score, axis=1)               # [B, S, 1]
    context= sum(attn * values, axis=1)           # [B, D]
    returns (context, attn)

Strategy: data-parallel over batch across 8 cores (4 batches/core).
Host-side prep (cheap marshalling):
  - qp = query @ W1 + b1 + b2 (tiny [32,512] matmul on host), transposed
    and chunked so it feeds the ScalarE activation bias port.
  - values cast to bf16 and transposed to [d, t] layout (the PE contracts
    over partitions, so the e-projection needs d on partitions).
  - values fp32 kept in natural [t, d] layout (pure reshape) for the
    context matmul (contracts over tokens).
  - bv dropped: softmax is invariant to a constant shift.

Device per core, per batch (streamed in 1024-token groups):
  - vpT[e,t] += W2[d,e].T @ valuesT[d,t]   (bf16 MACs, fp32 accum)
  - hiddenT = tanh(vpT + qp[e]) fused on ScalarE (evacuates PSUM)
  - score[1,t] += V[e].T @ hiddenT          (PE, V stationary)
  - rows = exp(score) on ScalarE; softmax sum + normalize on VectorE
  - attn cols regathered [128,32] via small SBUF->SBUF DMA
  - ctx[1,d] += attn_col.T @ values_nat     (float32r, full PE rate)
"""

import sys

if "/opt/trn_rl_repo" not in sys.path:
    sys.path.insert(0, "/opt/trn_rl_repo")

import numpy as np
import ml_dtypes

import concourse.bass as bass
import concourse.tile as tile
import concourse.mybir as mybir
from concourse import bacc
from concourse.bass import ds, ts
from concourse.bass_utils import run_bass_kernel_spmd

N_CORES = 8
B, S, D = 32, 4096, 512
NB = B // N_CORES          # batches per core = 4
NCHUNK = D // 128          # 4 chunks of the d/e axis
NGROUP = 4                 # token groups per batch
GT = S // NGROUP           # tokens per group = 1024
NJ = 32                    # ctx accumulation columns (token = 32*p + j)

F32 = mybir.dt.float32
F32R = mybir.dt.float32r
BF16 = mybir.dt.bfloat16

_NC_CACHE = {}


def build_core_kernel():
    """Bass module for one core: NB batches, S tokens, D dims."""
    nc = bacc.Bacc(None, target_bir_lowering=False, debug=False)

    valsT = nc.dram_tensor("valsT", [NB, NCHUNK, 128, S], BF16, kind="ExternalInput")
    vals = nc.dram_tensor("vals", [NB, 128, NJ, D], BF16, kind="ExternalInput")
    w2 = nc.dram_tensor("w2", [128, NCHUNK, D], BF16, kind="ExternalInput")
    vvec = nc.dram_tensor("vvec", [128, NCHUNK], BF16, kind="ExternalInput")
    qpt = nc.dram_tensor("qpt", [128, NCHUNK, NB], F32, kind="ExternalInput")

    attn_out = nc.dram_tensor("attn_out", [NB, S], F32, kind="ExternalOutput")
    ctx_out = nc.dram_tensor("ctx_out", [NB, D], F32, kind="ExternalOutput")

    with tile.TileContext(nc) as tc:
        with (
            tc.tile_pool(name="consts", bufs=1) as consts,
            tc.tile_pool(name="valT", bufs=3) as valT_pool,
            tc.tile_pool(name="hid", bufs=4) as hid_pool,
            tc.tile_pool(name="rows", bufs=2) as rows_pool,
            tc.tile_pool(name="small", bufs=4) as small_pool,
            tc.tile_pool(name="cols", bufs=2) as cols_pool,
            tc.tile_pool(name="nat", bufs=3) as nat_pool,
            tc.tile_pool(name="vp_ps", bufs=2, space="PSUM") as vp_ps_pool,
            tc.tile_pool(name="score_ps", bufs=1, space="PSUM") as score_ps_pool,
            tc.tile_pool(name="ctx_ps", bufs=1, space="PSUM") as ctx_ps_pool,
        ):
            w2_sb = consts.tile([128, NCHUNK, D], BF16)
            nc.sync.dma_start(w2_sb[:], w2.ap())
            v_sb = consts.tile([128, NCHUNK], BF16)
            nc.sync.dma_start(v_sb[:], vvec.ap())
            qpt_sb = consts.tile([128, NCHUNK, NB], F32)
            nc.sync.dma_start(qpt_sb[:], qpt.ap())

            for b in range(NB):
                rows = rows_pool.tile([1, S], F32, tag="rows")
                for g in range(NGROUP):
                    valT_g = valT_pool.tile([128, NCHUNK, GT], BF16, tag="valT")
                    nc.sync.dma_start(
                        valT_g[:],
                        valsT.ap()[b].rearrange("c p t -> p c t")[:, :, ds(g * GT, GT)],
                    )
                    score_ps = score_ps_pool.tile([1, GT], F32, tag="score")
                    for ce in range(NCHUNK):
                        vp_ps = vp_ps_pool.tile([128, GT], F32, tag="vp")
                        for ck in range(NCHUNK):
                            for n in range(GT // 512):
                                nc.tensor.matmul(
                                    vp_ps[:, ds(n * 512, 512)],
                                    w2_sb[:, ck, ds(ce * 128, 128)],
                                    valT_g[:, ck, ds(n * 512, 512)],
                                    start=(ck == 0),
                                    stop=(ck == NCHUNK - 1),
                                )
                        hid = hid_pool.tile([128, GT], BF16, tag="hid")
                        nc.scalar.activation(
                            hid[:],
                            vp_ps[:],
                            mybir.ActivationFunctionType.Tanh,
                            bias=qpt_sb[:, ce, ds(b, 1)],
                            scale=1.0,
                        )
                        for n in range(GT // 512):
                            nc.tensor.matmul(
                                score_ps[0:1, ds(n * 512, 512)],
                                v_sb[:, ds(ce, 1)],
                                hid[:, ds(n * 512, 512)],
                                start=(ce == 0),
                                stop=(ce == NCHUNK - 1),
                            )
                    nc.scalar.activation(
                        rows[0:1, ds(g * GT, GT)],
                        score_ps[0:1, :],
                        mybir.ActivationFunctionType.Exp,
                    )
                # softmax denominator and normalization (all on partition 0)
                ssum = small_pool.tile([1, 1], F32, tag="ssum")
                nc.vector.tensor_reduce(
                    ssum[0:1, :], rows[0:1, :],
                    axis=mybir.AxisListType.X, op=mybir.AluOpType.add,
                )
                recip = small_pool.tile([1, 1], F32, tag="recip")
                nc.vector.reciprocal(recip[0:1, :], ssum[0:1, :])
                rows_n = rows_pool.tile([1, S], F32, tag="rows_n")
                nc.vector.tensor_scalar_mul(
                    rows_n[0:1, :], rows[0:1, :], recip[0:1, 0:1]
                )
                nc.sync.dma_start(attn_out.ap()[ds(b, 1)], rows_n[0:1, :])
                # regather normalized attn as columns: cols[p, j] = attn[32p+j]
                # (gpsimd DMA casts fp32 -> bf16 during the gather)
                cols = cols_pool.tile([128, NJ], BF16, tag="cols")
                nc.gpsimd.dma_start(
                    cols[:], rows_n[0:1, :].rearrange("one (p j) -> one p j", j=NJ)
                )
                # context: ctx[1, d] += cols[:, j].T @ vals[128, j, :]
                ctx_ps = ctx_ps_pool.tile([1, D], F32, tag="ctx")
                for k in range(NJ // 8):
                    nat = nat_pool.tile([128, 8, D], BF16, tag="nat")
                    nc.sync.dma_start(nat[:], vals.ap()[b][:, ds(k * 8, 8), :])
                    for jj in range(8):
                        j = k * 8 + jj
                        nc.tensor.matmul(
                            ctx_ps[0:1, :],
                            cols[:, ds(j, 1)],
                            nat[:, jj, :],
                            start=(j == 0),
                            stop=(j == NJ - 1),
                        )
                ctx_sb = small_pool.tile([1, D], F32, tag="ctx_sb")
                nc.vector.tensor_copy(ctx_sb[0:1, :], ctx_ps[0:1, :])
                nc.sync.dma_start(ctx_out.ap()[ds(b, 1)], ctx_sb[0:1, :])

    nc.compile()
    return nc


def _to_bf16(x):
    return x.astype(ml_dtypes.bfloat16)


def prep_inputs(query, values, W1, b1, W2, b2, V, bv):
    """Host-side marshalling: shard over cores + layout/cast prep."""
    query = np.asarray(query, np.float32)
    values = np.asarray(values, np.float32)
    W1 = np.asarray(W1, np.float32)
    b1 = np.asarray(b1, np.float32)
    W2 = np.asarray(W2, np.float32)
    b2 = np.asarray(b2, np.float32)
    V = np.asarray(V, np.float32)

    qp = query @ W1 + b1 + b2                     # [B, D]
    w2b = np.ascontiguousarray(
        _to_bf16(W2).reshape(NCHUNK, 128, D).transpose(1, 0, 2)
    )                                             # [128, ck, D]
    vb = np.ascontiguousarray(_to_bf16(V[:, 0]).reshape(NCHUNK, 128).T)  # [128, ce]

    vbf = _to_bf16(values)                        # [B, S, D]
    in_maps = []
    for c in range(N_CORES):
        sl = slice(c * NB, (c + 1) * NB)
        valsT_c = np.ascontiguousarray(
            vbf[sl].reshape(NB, S, NCHUNK, 128).transpose(0, 2, 3, 1)
        )                                         # [NB, ck, 128, S]
        vals_c = np.ascontiguousarray(vbf[sl]).reshape(NB, 128, NJ, D)
        qpt_c = np.ascontiguousarray(
            qp[sl].reshape(NB, NCHUNK, 128).transpose(2, 1, 0)
        )                                         # [128, ce, NB]
        in_maps.append({
            "valsT": valsT_c,
            "vals": vals_c,
            "w2": w2b,
            "vvec": vb,
            "qpt": qpt_c,
        })
    return in_maps


def get_nc():
    if "nc" not in _NC_CACHE:
        _NC_CACHE["nc"] = build_core_kernel()
    return _NC_CACHE["nc"]


def kernel(**inputs):
    nc = get_nc()
    in_maps = prep_inputs(**inputs)
    res = run_bass_kernel_spmd(nc, in_maps, core_ids=list(range(N_CORES)))
    context = np.concatenate([r["ctx_out"] for r in res.results], axis=0)
    attn = np.concatenate([r["attn_out"] for r in res.results], axis=0)
    return context, attn.reshape(B, S, 1)
